# revision 17
# baseline (speedup 1.0000x reference)
"""Trainium2 Bass kernel for nn_BDFM_Multi (B=8,C=256,H=W=128,N=4).

Data-parallel over batch: one batch element per NeuronCore (8 cores).

Per-core computation (feature f [C,HW], m [N,H,W], HW=16384):
  z    = (m > 0.3)                                  binary
  er   = 13-tap separable min-filter(z), dl = 13-tap separable max-filter(z)
         (composition of 4 iters of 4x4 cv2-style erode/dilate)
         -> computed as banded 0/1 matmuls + thresholds (exact on binary data)
  fbu  = per-class channels (er, 1-dl, dl-er)       [12, HW]
  mid  = fbu @ f^T                                  [12, C]
  A'   = Wo2' @ mid^T                               [C, 12]
  G    = A' @ mid                                   [C, C]
  Wc   = Wo1' + G @ Wf'                             [C, C]  <- key collapse:
         out = Wo1'@f + G@(Wf'@f + beta_f 1^T) + beta_o 1^T
             = Wc @ f + u 1^T,   u = G @ beta_f + beta_o
  out  = Wc @ f + u                                 [C, HW]
  (exact algebraic refactor of out = BN(Wo @ [f; mid^T @ (mid @ BN(Wf@f))]))

All big matmuls run in float32r (~2-4e-4 rel err, full PE rate).
"""
import numpy as np
from contextlib import ExitStack

import concourse.bass as bass
import concourse.mybir as mybir
import concourse.tile as tile
from concourse import bacc
from concourse import bass_utils
from concourse.masks import make_identity

F32 = mybir.dt.float32
F32R = mybir.dt.float32r
ALU = mybir.AluOpType
ACTF = mybir.ActivationFunctionType

B, C, H, W, N = 8, 256, 128, 128, 4
HW = H * W
EPS = 1e-5
P = 128
PT = 512              # p-tile width for pass 2
NPT = HW // PT        # 32 p-tiles
G1 = 4                # h-chunks per transpose group in pass 1

_NC_CACHE = {}


def _band_consts():
    idx = np.arange(P)
    # erosion: output i covers input [i-8, i+4]; dilation: [i-4, i+8]
    band_er = ((idx[:, None] >= idx[None, :] - 8) &
               (idx[:, None] <= idx[None, :] + 4)).astype(np.float32)
    band_dl = ((idx[:, None] >= idx[None, :] - 4) &
               (idx[:, None] <= idx[None, :] + 8)).astype(np.float32)
    cnt_er = band_er.sum(axis=0, dtype=np.float32).reshape(P, 1)
    return band_er, band_dl, cnt_er


# layout of the packed fp32r parameter tensor "pk" [128, 1280]:
#   [0:512)     wo2t   : Wo2'^T chunks   [c-chunk ci -> cols ci*256:(ci+1)*256]
#   [512:1024)  wf_n   : Wf' blocks      [(ci*2+a)*128 ...] = Wf'[ci*128:,a*128:]
#   [1024:1152) band_er
#   [1152:1280) band_dl
#   [1280:1408) identity
PK_W = 1408


def build():
    if "nc" in _NC_CACHE:
        return _NC_CACHE["nc"]
    nc = bacc.Bacc(trn_type="TRN2", target_bir_lowering=False, debug=False)

    feature = nc.dram_tensor("feature", [C, HW], F32, kind="ExternalInput")
    pk = nc.dram_tensor("pk", [P, PK_W], F32, kind="ExternalInput")
    pkf = nc.dram_tensor("pkf", [P, 1029], F32, kind="ExternalInput")
    out = nc.dram_tensor("out", [C, HW], F32, kind="ExternalOutput")

    with tile.TileContext(nc) as tc, ExitStack() as ctx:
        persist = ctx.enter_context(tc.tile_pool(name="persist", bufs=1))

        # ---------------- loads ----------------
        # single fp32r cast-load for all packed params (first on the SWDGE queue)
        pk_sb = persist.tile([P, PK_W], F32R)
        nc.gpsimd.dma_start(out=pk_sb[:], in_=pk[:])
        wo2_sb = pk_sb[:, 0:512]
        wfn_sb = pk_sb[:, 512:1024]
        bander_sb = pk_sb[:, 1024:1152]
        banddl_sb = pk_sb[:, 1152:1280]
        ident_r = pk_sb[:, 1280:1408]

        pkf_sb = persist.tile([P, 1029], F32)
        nc.sync.dma_start(out=pkf_sb[:], in_=pkf[:])
        m_sb = pkf_sb[:, 0:512]
        wo1_sb = pkf_sb[:, 512:1024]
        betaf_sb = pkf_sb[:, 1024:1026]
        betao_sb = pkf_sb[:, 1026:1028]
        cnt_sb = pkf_sb[:, 1028:1029]

        # big feature load on the SWDGE queue, blk-interleaved so pass-1
        # h-chunks unblock as early as possible
        feat = persist.tile([P, 2 * HW], F32R)   # c-blk0 | c-blk1, cast fp32->fp32r
        # graduated chunk sizes: small first so pass-1 starts early, large later
        # for transfer efficiency (SWDGE transfers serialize across the 16 SDMA
        # engines, so chunk k completes only after all previous chunks)
        edges = [0, 512, 1024, 2048, 4096, 8192, 12288, 16384]
        for q in range(len(edges) - 1):
            for blk in range(2):
                nc.gpsimd.dma_start(
                    out=feat[:, blk * HW + edges[q]: blk * HW + edges[q + 1]],
                    in_=feature[blk * P:(blk + 1) * P, edges[q]:edges[q + 1]])

        # persistent results of phase 1
        fbuT = persist.tile([P, P * 12], F32R)     # [w, h*12 + k]
        wc_sb = persist.tile([P, 512], F32R)       # Wc^T blocks (a*2+o)
        u_sb = persist.tile([P, 2], F32)           # bias per o-blk

        wsrc = persist.tile([P, 512], F32)
        z_sb = persist.tile([P, N * P], F32R)

        # ---------------- morphology ----------------
        with tc.tile_pool(name="morph", bufs=1) as mo, \
             tc.tile_pool(name="morph_ps", bufs=2, space="PSUM") as mops:
            # PE warm-keeper: zero-dependency fp32 dummy matmuls that bridge the
            # startup window, so the HAM clock-gate is at 8/8 when real work
            # starts (an idle window >3.4us re-throttles the PE to 1.2 GHz).
            nc.vector.memset(wsrc[:], 0.0)
            wp = mops.tile([P, 512], F32, tag="mps")
            for i in range(10):
                nc.tensor.matmul(wp[:], wsrc[:, 0:P], wsrc[:],
                                 start=True, stop=True, skip_group_check=True)
            nc.vector.tensor_scalar(z_sb[:], m_sb, 0.3, None, op0=ALU.is_gt)

            ps_rows_er = mops.tile([P, N * P], F32, tag="mps")
            nc.tensor.matmul(ps_rows_er[:], bander_sb, z_sb[:],
                             start=True, stop=True)
            ps_rows_dl = mops.tile([P, N * P], F32, tag="mps")
            nc.tensor.matmul(ps_rows_dl[:], banddl_sb, z_sb[:],
                             start=True, stop=True)

            rows_er = mo.tile([P, N * P], F32R)
            nc.vector.tensor_scalar(rows_er[:], ps_rows_er[:], cnt_sb, None,
                                    op0=ALU.is_equal)
            rows_dl = mo.tile([P, N * P], F32R)
            nc.vector.tensor_scalar(rows_dl[:], ps_rows_dl[:], 0.5, None,
                                    op0=ALU.is_gt)

            # transpose each class tile -> [w, h]
            rows_erT = mo.tile([P, N * P], F32R)
            rows_dlT = mo.tile([P, N * P], F32R)
            for n in range(N):
                ps_tr = mops.tile([P, 2 * P], F32R, tag="mps")
                nc.tensor.matmul(ps_tr[:, 0:P], rows_er[:, n * P:(n + 1) * P],
                                 ident_r, is_transpose=True)
                nc.tensor.matmul(ps_tr[:, P:2 * P], rows_dl[:, n * P:(n + 1) * P],
                                 ident_r, is_transpose=True)
                nc.vector.tensor_copy(rows_erT[:, n * P:(n + 1) * P], ps_tr[:, 0:P])
                nc.vector.tensor_copy(rows_dlT[:, n * P:(n + 1) * P], ps_tr[:, P:2 * P])

            ps_cols_er = mops.tile([P, N * P], F32, tag="mps")
            nc.tensor.matmul(ps_cols_er[:], bander_sb, rows_erT[:],
                             start=True, stop=True)
            ps_cols_dl = mops.tile([P, N * P], F32, tag="mps")
            nc.tensor.matmul(ps_cols_dl[:], banddl_sb, rows_dlT[:],
                             start=True, stop=True)

            er_t = mo.tile([P, N * P], F32)   # er^T per class [w, h]
            dl_t = mo.tile([P, N * P], F32)
            nc.vector.tensor_scalar(er_t[:], ps_cols_er[:], cnt_sb, None,
                                    op0=ALU.is_equal)
            nc.vector.tensor_scalar(dl_t[:], ps_cols_dl[:], 0.5, None, op0=ALU.is_gt)

            # write channels into fbuT at [w, h*12 + k], k = 3n+j
            fbuT_v = fbuT.rearrange("w (h k) -> w h k", k=12)
            for n in range(N):
                src_er = er_t[:, n * P:(n + 1) * P]
                src_dl = dl_t[:, n * P:(n + 1) * P]
                nc.vector.tensor_copy(fbuT_v[:, :, 3 * n], src_er)
                nc.vector.tensor_scalar(fbuT_v[:, :, 3 * n + 1], src_dl, 0.0, None,
                                        op0=ALU.is_equal)
                nc.vector.tensor_tensor(fbuT_v[:, :, 3 * n + 2], src_dl, src_er,
                                        op=ALU.subtract)

        # ---------------- pass 1: feature transpose + mid ----------------
        mid_r = persist.tile([12, 256], F32R)
        with tc.tile_pool(name="mid_ps", bufs=1, space="PSUM") as midps, \
             tc.tile_pool(name="p1_ps", bufs=3, space="PSUM") as p1ps, \
             tc.tile_pool(name="p1_sb", bufs=4) as p1sb:
            mid_ps = midps.tile([12, 256], F32)
            for g in range(P // G1):
                tr = p1ps.tile([P, G1 * 256], F32R, tag="tr")
                for j in range(G1):
                    h = g * G1 + j
                    nc.tensor.matmul(tr[:, j * 256:j * 256 + P],
                                     feat[:, h * P:(h + 1) * P],
                                     ident_r, is_transpose=True)
                    nc.tensor.matmul(tr[:, j * 256 + P:(j + 1) * 256],
                                     feat[:, HW + h * P:HW + (h + 1) * P],
                                     ident_r, is_transpose=True)
                ft = p1sb.tile([P, G1 * 256], F32R, tag="ft")
                if g % 2 == 0:
                    nc.vector.tensor_copy(ft[:], tr[:])
                else:
                    nc.scalar.copy(ft[:], tr[:])
                for j in range(G1):
                    h = g * G1 + j
                    nc.tensor.matmul(mid_ps[:], fbuT[:, h * 12:h * 12 + 12],
                                     ft[:, j * 256:(j + 1) * 256],
                                     start=(h == 0), stop=(h == P - 1),
                                     skip_group_check=True)
            nc.vector.tensor_copy(mid_r[:], mid_ps[:])

        # ---------------- small stage: mid^T, A'^T, G^T, Wc, u ----------------
        with tc.tile_pool(name="sm_ps", bufs=1, space="PSUM") as smps, \
             tc.tile_pool(name="sm_sb", bufs=1) as smsb:
            # mid^T via PE transpose of [12,128] chunks (fp32r)
            ps_mt = smps.tile([P, 24], F32R, tag="mt")
            for ci in range(2):
                nc.tensor.matmul(ps_mt[:, ci * 12:(ci + 1) * 12],
                                 mid_r[:, ci * P:(ci + 1) * P],
                                 pk_sb[0:12, 1280:1292], is_transpose=True)
            mid_t = smsb.tile([P, 24], F32R)
            nc.vector.tensor_copy(mid_t[:], ps_mt[:])

            # A'^T = mid @ Wo2'^T   [12, 256]
            ps_at = smps.tile([12, 256], F32, tag="at")
            nc.tensor.matmul(ps_at[:], mid_t[:, 0:12], wo2_sb[:, 0:256],
                             start=True, stop=False)
            nc.tensor.matmul(ps_at[:], mid_t[:, 12:24], wo2_sb[:, 256:512],
                             start=False, stop=True)
            a_t = smsb.tile([12, 256], F32R)
            nc.vector.tensor_copy(a_t[:], ps_at[:])

            # G^T[c, o] = sum_k mid[k, c] A'^T[k, o];  chunks ci on partitions
            ps_gt = smps.tile([P, 512], F32, tag="gt")
            for ci in range(2):
                nc.tensor.matmul(ps_gt[:, ci * 256:(ci + 1) * 256],
                                 mid_r[:, ci * P:(ci + 1) * P], a_t[:],
                                 start=True, stop=True)
            gt_r = smsb.tile([P, 512], F32R)
            nc.vector.tensor_copy(gt_r[:], ps_gt[:])
            gt_f = smsb.tile([P, 512], F32)
            nc.vector.tensor_copy(gt_f[:], ps_gt[:])

            # X = Wf'^T @ G^T (= (G Wf')^T); blocks a (c_in chunk) on partitions
            ps_x = smps.tile([P, 512], F32, tag="x")
            for a in range(2):
                for ci in range(2):
                    nc.tensor.matmul(ps_x[:, a * 256:(a + 1) * 256],
                                     wfn_sb[:, (ci * 2 + a) * P:(ci * 2 + a + 1) * P],
                                     gt_r[:, ci * 256:(ci + 1) * 256],
                                     start=(ci == 0), stop=(ci == 1),
                                     skip_group_check=True)
            # Wc^T = Wo1'^T + X  (blocks (a*2+o) align with [a*256 + o*128])
            for a in range(2):
                nc.vector.tensor_tensor(wc_sb[:, a * 256:(a + 1) * 256],
                                        ps_x[:, a * 256:(a + 1) * 256],
                                        wo1_sb[:, a * 256:(a + 1) * 256],
                                        op=ALU.add)

            # u = G @ beta_f + beta_o   per o-blk  (fp32 matmuls)
            for o in range(2):
                ps_u = smps.tile([P, 1], F32, tag="u")
                nc.tensor.matmul(ps_u[:], gt_f[:, o * P:(o + 1) * P],
                                 betaf_sb[:, 0:1], start=True, stop=False)
                nc.tensor.matmul(ps_u[:], gt_f[:, 256 + o * P:256 + (o + 1) * P],
                                 betaf_sb[:, 1:2], start=False, stop=True)
                nc.scalar.activation(u_sb[:, o:o + 1], ps_u[:], ACTF.Identity,
                                     bias=betao_sb[:, o:o + 1])

        # ---------------- pass 2: out = Wc @ f + u ----------------
        with tc.tile_pool(name="out_ps", bufs=4, space="PSUM") as outps, \
             tc.tile_pool(name="p2_sb", bufs=2) as p2sb:
            for tg in range(NPT // 4):
                ot0 = p2sb.tile([P, 4 * PT], F32, tag="ot0")
                ot1 = p2sb.tile([P, 4 * PT], F32, tag="ot1")
                for tt in range(4):
                    t = tg * 4 + tt
                    c0 = t * PT
                    out_ps = outps.tile([P, 2 * PT], F32, tag="ops")
                    for o in range(2):
                        ops = out_ps[:, o * PT:(o + 1) * PT]
                        nc.tensor.matmul(ops,
                                         wc_sb[:, (0 * 2 + o) * P:(0 * 2 + o + 1) * P],
                                         feat[:, c0:c0 + PT],
                                         start=True, stop=False, skip_group_check=True)
                        nc.tensor.matmul(ops,
                                         wc_sb[:, (1 * 2 + o) * P:(1 * 2 + o + 1) * P],
                                         feat[:, HW + c0:HW + c0 + PT],
                                         start=False, stop=True, skip_group_check=True)
                    nc.scalar.activation(ot0[:, tt * PT:(tt + 1) * PT],
                                         out_ps[:, 0:PT],
                                         ACTF.Identity, bias=u_sb[:, 0:1])
                    nc.vector.tensor_scalar(ot1[:, tt * PT:(tt + 1) * PT],
                                            out_ps[:, PT:2 * PT],
                                            u_sb[:, 1:2], None, op0=ALU.add)
                g0 = tg * 4 * PT
                nc.sync.dma_start(out=out[0:P, g0:g0 + 4 * PT], in_=ot0[:])
                nc.gpsimd.dma_start(out=out[P:C, g0:g0 + 4 * PT], in_=ot1[:])

    nc.compile()
    _NC_CACHE["nc"] = nc
    return nc


def prepare_in_maps(feature, m, W_f, g_f, b_f, mu_f, v_f, W_o, g_o, b_o, mu_o, v_o):
    feature = np.asarray(feature, dtype=np.float32)
    m = np.asarray(m, dtype=np.float32)
    W_f = np.asarray(W_f, dtype=np.float32)
    W_o = np.asarray(W_o, dtype=np.float32)
    g_f, b_f, mu_f, v_f = (np.asarray(x, dtype=np.float32) for x in (g_f, b_f, mu_f, v_f))
    g_o, b_o, mu_o, v_o = (np.asarray(x, dtype=np.float32) for x in (g_o, b_o, mu_o, v_o))

    inv_f = g_f / np.sqrt(v_f + EPS)
    beta_f_v = b_f - mu_f * inv_f
    inv_o = g_o / np.sqrt(v_o + EPS)
    beta_o_v = b_o - mu_o * inv_o
    Wf_p = (inv_f[:, None] * W_f).astype(np.float32)          # [C, C]
    Wo1_p = (inv_o[:, None] * W_o[:, :C]).astype(np.float32)  # [C, C]
    Wo2_p = (inv_o[:, None] * W_o[:, C:]).astype(np.float32)  # [C, C]

    def blocks_t(Wp):
        # lhsT layout: blocks ci*2+o of Wp^T
        a = np.empty((P, 512), np.float32)
        for ci in range(2):
            for o in range(2):
                a[:, (ci * 2 + o) * P:(ci * 2 + o + 1) * P] = \
                    Wp[o * P:(o + 1) * P, ci * P:(ci + 1) * P].T
        return a

    def blocks_n(Wp):
        # natural-layout blocks ci*2+a: Wp[ci*128:(ci+1)*128, a*128:(a+1)*128]
        a_ = np.empty((P, 512), np.float32)
        for ci in range(2):
            for a in range(2):
                a_[:, (ci * 2 + a) * P:(ci * 2 + a + 1) * P] = \
                    Wp[ci * P:(ci + 1) * P, a * P:(a + 1) * P]
        return a_

    band_er, band_dl, cnt_er = _band_consts()
    pk = np.empty((P, PK_W), np.float32)
    pk[:, 0:512] = np.concatenate([Wo2_p.T[0:P, :], Wo2_p.T[P:C, :]], axis=1)
    pk[:, 512:1024] = blocks_n(Wf_p)
    pk[:, 1024:1152] = band_er
    pk[:, 1152:1280] = band_dl
    pk[:, 1280:1408] = np.eye(P, dtype=np.float32)

    pkf = np.empty((P, 1029), np.float32)
    pkf[:, 512:1024] = blocks_t(Wo1_p)
    pkf[:, 1024:1026] = beta_f_v.reshape(2, P).T
    pkf[:, 1026:1028] = beta_o_v.reshape(2, P).T
    pkf[:, 1028:1029] = cnt_er

    in_maps = []
    for b in range(B):
        im = {"pk": pk}
        pkf_b = pkf.copy()
        # m per class into columns [n*128:(n+1)*128]
        pkf_b[:, 0:512] = np.transpose(m[b], (1, 0, 2)).reshape(P, 512)
        im["pkf"] = pkf_b
        im["feature"] = np.ascontiguousarray(feature[b].reshape(C, HW))
        in_maps.append(im)
    return in_maps


def kernel(feature, m, W_f, g_f, b_f, mu_f, v_f, W_o, g_o, b_o, mu_o, v_o):
    nc = build()
    in_maps = prepare_in_maps(feature, m, W_f, g_f, b_f, mu_f, v_f,
                              W_o, g_o, b_o, mu_o, v_o)
    res = bass_utils.run_bass_kernel_spmd(nc, in_maps, list(range(B)))
    out = np.empty((B, C, H, W), np.float32)
    for b in range(B):
        out[b] = res.results[b]["out"].reshape(C, H, W)
    return out
